# revision 4
# baseline (speedup 1.0000x reference)
"""NVFP4 (E2M1, block-16) dequant matmul on 8 TRN2 NeuronCores — v2.

out[m, n] = sum_k (LUT[x[m,k]] * xs[m,k//16] * gx) * (LUT[w[n,k]] * ws[n,k//16] * gw) + bias[n]

Sharding: tensor-parallel along N: each of the 8 cores owns 1024 output
columns (weight/weight_scale/bias rows); x replicated.

v2 design (vs v1):
  - All weight code tiles land as raw fp8 (exact fp4 values) via the two
    HWDGE rings (sync + scalar queues), freeing GpSimd from DMA descriptor
    generation and halving SBUF-side fabric traffic vs v1's cast-DMAs.
  - Deep prefetch: every weight group gets a dedicated SBUF tile, so the
    HBM stream runs flat-out, decoupled from compute.
  - The 8.4M-element dequant multiply (wh = wv * wsT) is split across
    three engines: DVE in-place bf16 TTs (13 groups), GpSimd mixed TTs
    (3 groups, chunk-granular for early start), with ScalarE doing the
    fp8->bf16 casts for 6 groups and SWDGE cast-DMA landing 6 more
    directly in bf16.
  - PE emission order interleaves GpSimd-group matmuls at their expected
    completion times (PSUM accumulation is order-independent).

Host-side marshaling stays format-only (LUT decode + layout + dtype cast);
all reference arithmetic (scale multiplies, matmul, bias) is on device.
"""

import json
from contextlib import ExitStack

import ml_dtypes
import numpy as np

import concourse.bass as bass
import concourse.mybir as mybir
import concourse.tile as tile
from concourse.bass_utils import run_bass_kernel_spmd


def _split_multi_waits(m: dict) -> dict:
    """This walrus build allows at most one sync-wait command per instruction.
    Hoist extra waits into standalone EventSemaphore instructions issued just
    before the owning instruction on the same engine queue (semantically
    identical: the engine stalls in order)."""
    for fn in m["functions"]:
        for blk in fn["blocks"]:
            new = []
            ctr = 0
            for inst in blk["instructions"]:
                si = inst.get("sync_info")
                waits = (si or {}).get("on_wait") or []
                if len(waits) > 1:
                    for w in waits[:-1]:
                        new.append({
                            "debug": inst.get("debug", 0),
                            "engine": inst["engine"],
                            "ins": [],
                            "outs": [],
                            "name": f"{inst['name']}-hw{ctr}",
                            "opcode": "EventSemaphore",
                            "sync_info": {"on_update": [], "on_wait": [w]},
                        })
                        ctr += 1
                    si["on_wait"] = [waits[-1]]
                new.append(inst)
            blk["instructions"] = new
    return m


class _SplitWaitBass(bass.Bass):
    def to_json_bytes(self) -> bytes:
        m = json.loads(super().to_json_bytes())
        return json.dumps(_split_multi_waits(m)).encode()


BF16 = ml_dtypes.bfloat16
FP4_LUT = np.array(
    [0.0, 0.5, 1.0, 1.5, 2.0, 3.0, 4.0, 6.0,
     -0.0, -0.5, -1.0, -1.5, -2.0, -3.0, -4.0, -6.0],
    dtype=np.float32,
)

M, K, N = 64, 8192, 8192
NCORES = 8
NS = N // NCORES        # 1024 output columns per core
BLOCK = 16
B = K // BLOCK          # 512 scale blocks along K
P = 128                 # partitions
CHUNKS = K // P         # 64 K-chunks
CB = B // P             # 4 scale-chunk columns (c index)
J = BLOCK               # 16 j-groups (one group = CB chunks = 512 rows)
GW = CB * NS            # 4096 columns per weight group tile

# ---- group role assignment (tunable) ----
G_GROUPS = (0, 1, 2)                    # GpSimd mixed TT (fp8 x bf16), chunked
D_GROUPS = (3,)                         # DVE mixed TT (fp8 x bf16) — rate probe
A_GROUPS = (4, 6, 8, 10, 12, 14)        # ScalarE cast + DVE in-place bf16 TT
S_GROUPS = (5, 7, 9, 11, 13, 15)        # SWDGE cast-DMA lands bf16 + DVE TT
# DVE-line consumption order (induces DVE/ACT queue order)
DVE_LINE = (4, 5, 6, 7, 8, 9, 10, 3, 11, 12, 13, 14, 15)
CHUNKED_DVE = (14, 15)                  # last groups: chunk-granular TT + MMs
# sync-ring DMA issue order (g0/g1/g2 are quarter-split)
SYNC_ORDER = (0, 4, 6, 1, 8, 10, 3, 12, 2, 14)

_CACHE: dict = {}


def _build_program() -> bass.Bass:
    nc = _SplitWaitBass("TRN2", target_bir_lowering=False, debug=False,
                        num_devices=NCORES)
    dt = mybir.dt

    wvp = nc.dram_tensor("wvp", [P, CHUNKS * NS], dt.float8e4,
                         kind="ExternalInput").ap()
    wst = nc.dram_tensor("wst", [P, GW], dt.bfloat16,
                         kind="ExternalInput").ap()
    xvp = nc.dram_tensor("xvp", [P, CHUNKS * M], dt.bfloat16,
                         kind="ExternalInput").ap()
    xst = nc.dram_tensor("xst", [P, CB * M], dt.bfloat16,
                         kind="ExternalInput").ap()
    gs = nc.dram_tensor("gs", [P, 2], dt.float32, kind="ExternalInput").ap()
    bia = nc.dram_tensor("bia", [1, NS], dt.bfloat16, kind="ExternalInput").ap()
    out = nc.dram_tensor("out", [M, NS], dt.bfloat16, kind="ExternalOutput").ap()

    with tile.TileContext(nc) as tc, ExitStack() as ctx:
        const = ctx.enter_context(tc.tile_pool(name="const", bufs=1))
        w8pool = ctx.enter_context(tc.tile_pool(name="w8", bufs=10))
        hpool = ctx.enter_context(tc.tile_pool(name="whA", bufs=4))
        spool = ctx.enter_context(tc.tile_pool(name="whS", bufs=6))
        gpool = ctx.enter_context(tc.tile_pool(name="whG", bufs=2))
        ppool = ctx.enter_context(tc.tile_pool(name="acc", bufs=1, space="PSUM"))

        NQ = NS  # 1024 columns per quarter of a group tile

        # ---- prologue DMAs ----
        # scalar (ACT/HWDGE) ring: small x-side tensors first
        gt = const.tile([P, 2], dt.float32)
        nc.scalar.dma_start(gt[:], gs[:])
        xsT = const.tile([P, CB * M], dt.bfloat16)
        nc.scalar.dma_start(xsT[:], xst[:])
        xva = const.tile([P, CHUNKS * M], dt.bfloat16)
        nc.scalar.dma_start(xva[:], xvp[:])
        bsb = const.tile([1, NS], dt.bfloat16)
        nc.scalar.dma_start(bsb[:], bia[:])

        # sync (SP/HWDGE) ring: weight scales (quartered) then weight groups
        wsT = const.tile([P, GW], dt.bfloat16)
        for q in range(CB):
            nc.sync.dma_start(wsT[:, q * NQ:(q + 1) * NQ],
                              wst[:, q * NQ:(q + 1) * NQ])
        wv8: dict = {}
        for g in SYNC_ORDER:
            wv = w8pool.tile([P, GW], dt.float8e4, tag=f"w8_{g}", name=f"w8_{g}", bufs=1)
            wv8[g] = wv
            gbase = g * GW
            if g in G_GROUPS:
                for q in range(CB):
                    nc.sync.dma_start(
                        wv[:, q * NQ:(q + 1) * NQ],
                        wvp[:, gbase + q * NQ: gbase + (q + 1) * NQ])
            else:
                nc.sync.dma_start(wv[:], wvp[:, gbase:gbase + GW])

        # SWDGE (gpsimd) ring: cast-DMAs land S-groups directly as bf16.
        # Emitted before the GpSimd TTs so descriptor generation happens
        # up-front on the Q7s.
        whS: dict = {}
        for g in S_GROUPS:
            wh = spool.tile([P, GW], dt.bfloat16, tag=f"whS_{g}", name=f"whS_{g}", bufs=1)
            whS[g] = wh
            nc.gpsimd.dma_start(wh[:], wvp[:, g * GW:(g + 1) * GW])

        # ---- x-side dequant ----
        gcol = const.tile([P, 1], dt.float32)
        nc.vector.tensor_mul(gcol[:], gt[:, 0:1], gt[:, 1:2])
        xsb = const.tile([P, CB * M], dt.bfloat16)
        nc.vector.tensor_scalar_mul(xsb[:], xsT[:], gcol[:])
        xsr = const.tile([P, 4 * CB * M], dt.bfloat16)      # [128, 1024]
        for r in range(4):
            sl = slice(r * CB * M, (r + 1) * CB * M)
            if r < 2:
                nc.scalar.copy(xsr[:, sl], xsb[:])
            else:
                nc.vector.tensor_copy(xsr[:, sl], xsb[:])
        xhat = const.tile([P, CHUNKS * M], dt.bfloat16)
        for r in range(4):
            sl = slice(r * 4 * CB * M, (r + 1) * 4 * CB * M)
            nc.vector.tensor_mul(xhat[:, sl], xva[:, sl], xsr[:])

        ones = const.tile([1, M], dt.bfloat16)
        nc.vector.memset(ones[:], 1.0)

        psum = ppool.tile([M, NS], dt.float32)

        # bias rides the first accumulation write (start=True clears PSUM)
        for h in range(2):
            nc.tensor.matmul(
                psum[:, h * 512:(h + 1) * 512],
                ones[:1, :],
                bsb[:1, h * 512:(h + 1) * 512],
                start=True,
                stop=False,
            )

        # ---- main pipeline ----
        # PE emission order: DVE-line groups with GpSimd chunk-TT matmuls
        # interleaved at their expected completion times. PSUM accumulation
        # is order-independent; stop rides the final emitted matmul.
        whG: dict = {}
        whA: dict = {}

        pe_seq: list = []           # ("g", g, c) one chunk | ("l", g) 4 chunks
        gq = [("g", g, c) for g in G_GROUPS for c in range(CB)]  # 12 chunk units
        line = list(DVE_LINE)
        # interleave: 1 gpsimd chunk per dve-line step, front-loaded g0
        mix = [gq[0], gq[1]]
        gi = 2
        for i, g in enumerate(line):
            mix.append(("l", g))
            if gi < len(gq) and i % 1 == 0 and gi - 2 < i + 1:
                mix.append(gq[gi])
                gi += 1
        while gi < len(gq):
            mix.append(gq[gi])
            gi += 1
        pe_seq = mix

        def emit_mms(g, c, wh, stop):
            t = g * CB + c
            for h in range(2):
                nc.tensor.matmul(
                    psum[:, h * 512:(h + 1) * 512],
                    xhat[:, t * M:(t + 1) * M],
                    wh[:, c * NS + h * 512: c * NS + (h + 1) * 512],
                    start=False,
                    stop=stop and (h == 1),
                )

        n_units = len(pe_seq)
        for ui, unit in enumerate(pe_seq):
            last_unit = ui == n_units - 1
            if unit[0] == "g":
                _, g, c = unit
                if c == 0:
                    whG[g] = gpool.tile([P, GW], dt.bfloat16, tag=f"whG_{g}", name=f"whG_{g}", bufs=1)
                wh = whG[g]
                cs = slice(c * NQ, (c + 1) * NQ)
                # GpSimd mixed TT: fp8 codes * bf16 scales -> bf16
                nc.gpsimd.tensor_mul(wh[:, cs], wv8[g][:, cs], wsT[:, cs])
                emit_mms(g, c, wh, stop=last_unit)
            else:
                _, g = unit
                if g in S_GROUPS:
                    wh = whS[g]
                    if g in CHUNKED_DVE:
                        for c in range(CB):
                            cs = slice(c * NQ, (c + 1) * NQ)
                            nc.vector.tensor_mul(wh[:, cs], wh[:, cs],
                                                 wsT[:, cs])
                            emit_mms(g, c, wh, stop=last_unit and c == CB - 1)
                    else:
                        nc.vector.tensor_mul(wh[:], wh[:], wsT[:])
                        for c in range(CB):
                            emit_mms(g, c, wh, stop=last_unit and c == CB - 1)
                elif g in A_GROUPS:
                    wh = hpool.tile([P, GW], dt.bfloat16, tag="whA", name=f"whA_{g}")
                    whA[g] = wh
                    nc.scalar.copy(wh[:], wv8[g][:])         # ACT fp8->bf16
                    if g in CHUNKED_DVE:
                        for c in range(CB):
                            cs = slice(c * NQ, (c + 1) * NQ)
                            nc.vector.tensor_mul(wh[:, cs], wh[:, cs],
                                                 wsT[:, cs])
                            emit_mms(g, c, wh, stop=last_unit and c == CB - 1)
                    else:
                        nc.vector.tensor_mul(wh[:], wh[:], wsT[:])
                        for c in range(CB):
                            emit_mms(g, c, wh, stop=last_unit and c == CB - 1)
                else:  # D group: DVE mixed TT straight from fp8
                    wh = hpool.tile([P, GW], dt.bfloat16, tag="whA", name=f"whD_{g}")
                    nc.vector.tensor_mul(wh[:], wv8[g][:], wsT[:])
                    for c in range(CB):
                        emit_mms(g, c, wh, stop=last_unit and c == CB - 1)

        # ---- tail: PSUM -> SBUF -> HBM ----
        osb = const.tile([M, NS], dt.bfloat16)
        nc.vector.tensor_copy(osb[:, 0:512], psum[:, 0:512])
        nc.scalar.copy(osb[:, 512:NS], psum[:, 512:NS])
        nc.scalar.dma_start(out[:], osb[:])

    return nc


def _perm_k(vals_2d: np.ndarray) -> np.ndarray:
    """[R, K] fp values -> [K, R] with K permuted as r = j*B + b."""
    r = vals_2d.shape[0]
    return (
        vals_2d.reshape(r, B, BLOCK).transpose(2, 1, 0).reshape(K, r)
    )


def _swz(rows_2d: np.ndarray, width: int) -> np.ndarray:
    """[n_chunks*128, width] -> [128, n_chunks*width]: row p holds chunk-major
    data for partition p (per-partition-contiguous DMA layout)."""
    n = rows_2d.shape[0] // P
    return np.ascontiguousarray(
        rows_2d.reshape(n, P, width).transpose(1, 0, 2).reshape(P, n * width)
    )


def prepare_in_maps(**inputs) -> list[dict[str, np.ndarray]]:
    x = np.asarray(inputs["x"]).astype(np.int64)
    xs = np.asarray(inputs["x_scale"], dtype=np.float32)
    gx = np.float32(np.asarray(inputs["x_global_scale"]).reshape(-1)[0])
    w = np.asarray(inputs["weight"]).astype(np.int64)
    ws = np.asarray(inputs["weight_scale"], dtype=np.float32)
    gw = np.float32(np.asarray(inputs["weight_global_scale"]).reshape(-1)[0])
    b = np.asarray(inputs["bias"], dtype=np.float32)

    FP8 = ml_dtypes.float8_e4m3
    xvp = _swz(_perm_k(FP4_LUT[x]).astype(BF16), M)                  # [128, 4096]
    xst = _swz(np.ascontiguousarray(xs.T), M).astype(BF16)           # [128, 256]
    gs = np.tile(np.array([[gx, gw]], dtype=np.float32), (P, 1))

    wv = FP4_LUT[w]                                                  # [N, K] f32
    in_maps = []
    for c in range(NCORES):
        sl = slice(c * NS, (c + 1) * NS)
        in_maps.append({
            "wvp": _swz(_perm_k(wv[sl]).astype(FP8), NS),            # [128, 64*NS]
            "wst": _swz(ws[sl].T.astype(BF16), NS),                  # [128, 4*NS]
            "xvp": xvp,
            "xst": xst,
            "gs": gs,
            "bia": np.ascontiguousarray(b[sl].reshape(1, NS)).astype(BF16),
        })
    return in_maps


LAST_RESULTS = None


def kernel(**inputs) -> np.ndarray:
    global LAST_RESULTS
    if "nc" not in _CACHE:
        _CACHE["nc"] = _build_program()
    nc = _CACHE["nc"]

    in_maps = prepare_in_maps(**inputs)
    res = run_bass_kernel_spmd(nc, in_maps, core_ids=list(range(NCORES)))
    LAST_RESULTS = res
    out = np.concatenate([res.results[c]["out"] for c in range(NCORES)], axis=1)
    return out.astype(BF16)


# revision 5
# speedup vs baseline: 1.0856x; 1.0856x over previous
"""NVFP4 (E2M1, block-16) dequant matmul on 8 TRN2 NeuronCores — v3.

out[m, n] = sum_k (LUT[x[m,k]] * xs[m,k//16] * gx) * (LUT[w[n,k]] * ws[n,k//16] * gw) + bias[n]

Sharding: tensor-parallel along N: each of the 8 cores owns 1024 output
columns (weight/weight_scale/bias rows); x replicated.

v3 design (measured-fact driven):
  - DVE tensor_tensor at 2x (bf16) is the ONLY efficient multiplier;
    GpSimd tensor ops halve both engines via the shared SBUF port, so
    GpSimd does zero tensor work. DVE runs every dequant multiply.
  - fp8->bf16 casts: ScalarE (ACT) for 8 groups, SWDGE cast-DMA for the
    other 8 (GpSimd Q7 only generates descriptors -> no port contention).
  - Weight TTs are fused in PAIRS [128, 2, 4096] with a stride-0
    broadcast wsT operand; x dequant is one broadcast TT (no replication).
  - dma_start instructions cost ~0.6-1.2us of sequencer issue time each,
    so DMAs are few and big, split across both HWDGE rings + SWDGE.
  - First/last groups are chunk-granular to shorten kernel head/tail.

Host-side marshaling stays format-only (LUT decode + layout + dtype cast);
all reference arithmetic (scale multiplies, matmul, bias) is on device.
"""

import json
from contextlib import ExitStack

import ml_dtypes
import numpy as np

import concourse.bass as bass
import concourse.mybir as mybir
import concourse.tile as tile
from concourse.bass_utils import run_bass_kernel_spmd


def _split_multi_waits(m: dict) -> dict:
    """This walrus build allows at most one sync-wait command per instruction.
    Hoist extra waits into standalone EventSemaphore instructions issued just
    before the owning instruction on the same engine queue (semantically
    identical: the engine stalls in order)."""
    for fn in m["functions"]:
        for blk in fn["blocks"]:
            new = []
            ctr = 0
            for inst in blk["instructions"]:
                si = inst.get("sync_info")
                waits = (si or {}).get("on_wait") or []
                if len(waits) > 1:
                    for w in waits[:-1]:
                        new.append({
                            "debug": inst.get("debug", 0),
                            "engine": inst["engine"],
                            "ins": [],
                            "outs": [],
                            "name": f"{inst['name']}-hw{ctr}",
                            "opcode": "EventSemaphore",
                            "sync_info": {"on_update": [], "on_wait": [w]},
                        })
                        ctr += 1
                    si["on_wait"] = [waits[-1]]
                new.append(inst)
            blk["instructions"] = new
    return m


class _SplitWaitBass(bass.Bass):
    def to_json_bytes(self) -> bytes:
        m = json.loads(super().to_json_bytes())
        return json.dumps(_split_multi_waits(m)).encode()


BF16 = ml_dtypes.bfloat16
FP4_LUT = np.array(
    [0.0, 0.5, 1.0, 1.5, 2.0, 3.0, 4.0, 6.0,
     -0.0, -0.5, -1.0, -1.5, -2.0, -3.0, -4.0, -6.0],
    dtype=np.float32,
)

M, K, N = 64, 8192, 8192
NCORES = 8
NS = N // NCORES        # 1024 output columns per core
BLOCK = 16
B = K // BLOCK          # 512 scale blocks along K
P = 128                 # partitions
CHUNKS = K // P         # 64 K-chunks
CB = B // P             # 4 scale-chunk columns (c index)
J = BLOCK               # 16 j-groups (one group = CB chunks = 512 rows)
GW = CB * NS            # 4096 columns per weight group tile
NQ = NS                 # 1024 columns per chunk of a group tile

# Roles: even groups = ACT cast (fp8 lands via HWDGE), odd = SWDGE cast-DMA.
# Line order: g0 (chunked), g1, pairs (2,3)..(12,13), g14 (chunked),
# g15 (chunked).
PAIRS = ((2, 3), (4, 5), (6, 7), (8, 9), (10, 11), (12, 13))

_CACHE: dict = {}


def _build_program() -> bass.Bass:
    nc = _SplitWaitBass("TRN2", target_bir_lowering=False, debug=False,
                        num_devices=NCORES)
    dt = mybir.dt

    wvp = nc.dram_tensor("wvp", [P, CHUNKS * NS], dt.float8e4,
                         kind="ExternalInput").ap()
    wst = nc.dram_tensor("wst", [P, GW], dt.bfloat16,
                         kind="ExternalInput").ap()
    xvp = nc.dram_tensor("xvp", [P, CHUNKS * M], dt.bfloat16,
                         kind="ExternalInput").ap()
    xst = nc.dram_tensor("xst", [P, CB * M], dt.bfloat16,
                         kind="ExternalInput").ap()
    gs = nc.dram_tensor("gs", [P, 2], dt.float32, kind="ExternalInput").ap()
    bia = nc.dram_tensor("bia", [1, NS], dt.bfloat16, kind="ExternalInput").ap()
    out = nc.dram_tensor("out", [M, NS], dt.bfloat16, kind="ExternalOutput").ap()

    with tile.TileContext(nc) as tc, ExitStack() as ctx:
        const = ctx.enter_context(tc.tile_pool(name="const", bufs=1))
        w8pool = ctx.enter_context(tc.tile_pool(name="w8", bufs=3))
        prpool = ctx.enter_context(tc.tile_pool(name="pair", bufs=3))
        endpool = ctx.enter_context(tc.tile_pool(name="ends", bufs=1))
        ppool = ctx.enter_context(tc.tile_pool(name="acc", bufs=1, space="PSUM"))

        # ---- DMAs ----
        # sync (SP/HWDGE) ring: weight scales (quartered for early TT start),
        # g0 codes (quartered), x codes, then half the even-group codes.
        wsT = const.tile([P, GW], dt.bfloat16)
        for q in range(CB):
            nc.sync.dma_start(wsT[:, q * NQ:(q + 1) * NQ],
                              wst[:, q * NQ:(q + 1) * NQ])
        wv8: dict = {}
        wv8[0] = w8pool.tile([P, GW], dt.float8e4, tag="w8", name="w8_0")
        for q in range(CB):
            nc.sync.dma_start(wv8[0][:, q * NQ:(q + 1) * NQ],
                              wvp[:, q * NQ:(q + 1) * NQ])
        xva = const.tile([P, CHUNKS * M], dt.bfloat16)
        nc.sync.dma_start(xva[:], xvp[:])
        for g in (2, 6, 10, 14):
            wv8[g] = w8pool.tile([P, GW], dt.float8e4, tag="w8",
                                 name=f"w8_{g}")
            nc.sync.dma_start(wv8[g][:], wvp[:, g * GW:(g + 1) * GW])

        # scalar (ACT/HWDGE) ring: small x-side tensors + other even groups
        gt = const.tile([P, 2], dt.float32)
        nc.scalar.dma_start(gt[:], gs[:])
        xsT = const.tile([P, CB * M], dt.bfloat16)
        nc.scalar.dma_start(xsT[:], xst[:])
        bsb = const.tile([1, NS], dt.bfloat16)
        nc.scalar.dma_start(bsb[:], bia[:])
        for g in (4, 8, 12):
            wv8[g] = w8pool.tile([P, GW], dt.float8e4, tag="w8",
                                 name=f"w8_{g}")
            nc.scalar.dma_start(wv8[g][:], wvp[:, g * GW:(g + 1) * GW])

        # dequantized-weight tiles: ends are single-group, middles paired
        whA0 = endpool.tile([P, GW], dt.bfloat16, name="whA0")
        whS1 = endpool.tile([P, GW], dt.bfloat16, name="whS1")
        whA14 = endpool.tile([P, GW], dt.bfloat16, name="whA14")
        pair_t: dict = {}
        for ge, go in PAIRS:
            pair_t[ge] = prpool.tile([P, 2 * GW], dt.bfloat16, tag="pair",
                                     name=f"pair_{ge}")
        whS15 = prpool.tile([P, 2 * GW], dt.bfloat16, tag="pair", name="whS15")

        # SWDGE (gpsimd Q7) ring: odd groups land directly as bf16.
        # Pair-ring slot semaphores self-pace the stream.
        nc.gpsimd.dma_start(whS1[:], wvp[:, 1 * GW:2 * GW])
        for ge, go in PAIRS:
            nc.gpsimd.dma_start(pair_t[ge][:, GW:2 * GW],
                                wvp[:, go * GW:(go + 1) * GW])
        nc.gpsimd.dma_start(whS15[:, 0:GW], wvp[:, 15 * GW:16 * GW])

        # ---- x-side dequant ----
        gcol = const.tile([P, 1], dt.float32)
        nc.vector.tensor_mul(gcol[:], gt[:, 0:1], gt[:, 1:2])
        xsb = const.tile([P, CB * M], dt.bfloat16)
        nc.vector.tensor_scalar_mul(xsb[:], xsT[:], gcol[:])
        # xhat[p, (j c m)] = xva * xsb[p, (c m)] broadcast over j
        xhat = const.tile([P, CHUNKS * M], dt.bfloat16)
        xsb_b = xsb[:].unsqueeze(1).broadcast_to([P, J, CB * M])
        nc.vector.tensor_mul(
            xhat[:].rearrange("p (j w) -> p j w", j=J),
            xva[:].rearrange("p (j w) -> p j w", j=J),
            xsb_b,
        )

        ones = const.tile([1, M], dt.bfloat16)
        nc.vector.memset(ones[:], 1.0)

        psum = ppool.tile([M, NS], dt.float32)
        for h in range(2):
            nc.tensor.matmul(
                psum[:, h * 512:(h + 1) * 512],
                ones[:1, :],
                bsb[:1, h * 512:(h + 1) * 512],
                start=True,
                stop=False,
            )

        def emit_mms(g, c, wh, col0, stop):
            """8 matmuls for group g chunk c; weight columns start at col0."""
            t = g * CB + c
            for h in range(2):
                nc.tensor.matmul(
                    psum[:, h * 512:(h + 1) * 512],
                    xhat[:, t * M:(t + 1) * M],
                    wh[:, col0 + c * NS + h * 512: col0 + c * NS + (h + 1) * 512],
                    start=False,
                    stop=stop,
                )

        wsT_b2 = wsT[:].unsqueeze(1).broadcast_to([P, 2, GW])

        # ---- g0: ACT cast + DVE TT, chunk-granular (early start) ----
        for c in range(CB):
            cs = slice(c * NQ, (c + 1) * NQ)
            nc.scalar.copy(whA0[:, cs], wv8[0][:, cs])
            nc.vector.tensor_mul(whA0[:, cs], whA0[:, cs], wsT[:, cs])
            emit_mms(0, c, whA0, 0, stop=False)

        # ---- g1: SWDGE-landed bf16, single whole TT ----
        nc.vector.tensor_mul(whS1[:], whS1[:], wsT[:])
        for c in range(CB):
            emit_mms(1, c, whS1, 0, stop=False)

        # ---- middle pairs: one fused TT per pair ----
        for ge, go in PAIRS:
            pt = pair_t[ge]
            nc.scalar.copy(pt[:, 0:GW], wv8[ge][:])          # ACT fp8->bf16
            nc.vector.tensor_mul(
                pt[:].rearrange("p (a w) -> p a w", a=2),
                pt[:].rearrange("p (a w) -> p a w", a=2),
                wsT_b2,
            )
            for c in range(CB):
                emit_mms(ge, c, pt, 0, stop=False)
            for c in range(CB):
                emit_mms(go, c, pt, GW, stop=False)

        # ---- g14: ACT cast + chunked TT ----
        nc.scalar.copy(whA14[:], wv8[14][:])
        for c in range(CB):
            cs = slice(c * NQ, (c + 1) * NQ)
            nc.vector.tensor_mul(whA14[:, cs], whA14[:, cs], wsT[:, cs])
            emit_mms(14, c, whA14, 0, stop=False)

        # ---- g15: SWDGE-landed bf16, chunked TT (short tail) ----
        for c in range(CB):
            cs = slice(c * NQ, (c + 1) * NQ)
            nc.vector.tensor_mul(whS15[:, cs], whS15[:, cs], wsT[:, cs])
            emit_mms(15, c, whS15, 0, stop=(c == CB - 1))

        # ---- tail: PSUM -> SBUF -> HBM ----
        osb = const.tile([M, NS], dt.bfloat16)
        nc.vector.tensor_copy(osb[:, 0:512], psum[:, 0:512])
        nc.scalar.copy(osb[:, 512:NS], psum[:, 512:NS])
        nc.scalar.dma_start(out[:], osb[:])

    return nc


def _perm_k(vals_2d: np.ndarray) -> np.ndarray:
    """[R, K] fp values -> [K, R] with K permuted as r = j*B + b."""
    r = vals_2d.shape[0]
    return (
        vals_2d.reshape(r, B, BLOCK).transpose(2, 1, 0).reshape(K, r)
    )


def _swz(rows_2d: np.ndarray, width: int) -> np.ndarray:
    """[n_chunks*128, width] -> [128, n_chunks*width]: row p holds chunk-major
    data for partition p (per-partition-contiguous DMA layout)."""
    n = rows_2d.shape[0] // P
    return np.ascontiguousarray(
        rows_2d.reshape(n, P, width).transpose(1, 0, 2).reshape(P, n * width)
    )


def prepare_in_maps(**inputs) -> list[dict[str, np.ndarray]]:
    x = np.asarray(inputs["x"]).astype(np.int64)
    xs = np.asarray(inputs["x_scale"], dtype=np.float32)
    gx = np.float32(np.asarray(inputs["x_global_scale"]).reshape(-1)[0])
    w = np.asarray(inputs["weight"]).astype(np.int64)
    ws = np.asarray(inputs["weight_scale"], dtype=np.float32)
    gw = np.float32(np.asarray(inputs["weight_global_scale"]).reshape(-1)[0])
    b = np.asarray(inputs["bias"], dtype=np.float32)

    FP8 = ml_dtypes.float8_e4m3
    xvp = _swz(_perm_k(FP4_LUT[x]).astype(BF16), M)                  # [128, 4096]
    xst = _swz(np.ascontiguousarray(xs.T), M).astype(BF16)           # [128, 256]
    gs = np.tile(np.array([[gx, gw]], dtype=np.float32), (P, 1))

    wv = FP4_LUT[w]                                                  # [N, K] f32
    in_maps = []
    for c in range(NCORES):
        sl = slice(c * NS, (c + 1) * NS)
        in_maps.append({
            "wvp": _swz(_perm_k(wv[sl]).astype(FP8), NS),            # [128, 64*NS]
            "wst": _swz(ws[sl].T.astype(BF16), NS),                  # [128, 4*NS]
            "xvp": xvp,
            "xst": xst,
            "gs": gs,
            "bia": np.ascontiguousarray(b[sl].reshape(1, NS)).astype(BF16),
        })
    return in_maps


LAST_RESULTS = None


def kernel(**inputs) -> np.ndarray:
    global LAST_RESULTS
    if "nc" not in _CACHE:
        _CACHE["nc"] = _build_program()
    nc = _CACHE["nc"]

    in_maps = prepare_in_maps(**inputs)
    res = run_bass_kernel_spmd(nc, in_maps, core_ids=list(range(NCORES)))
    LAST_RESULTS = res
    out = np.concatenate([res.results[c]["out"] for c in range(NCORES)], axis=1)
    return out.astype(BF16)


# revision 7
# speedup vs baseline: 1.2637x; 1.1640x over previous
"""NVFP4 (E2M1, block-16) dequant matmul on 8 TRN2 NeuronCores — v4.

out[m, n] = sum_k (LUT[x[m,k]] * xs[m,k//16] * gx) * (LUT[w[n,k]] * ws[n,k//16] * gw) + bias[n]

Sharding: tensor-parallel along N: each of the 8 cores owns 1024 output
columns (weight/weight_scale/bias rows); x replicated.

v4 design (measured-fact driven):
  - DVE tensor_tensor at 2x (bf16) is the only efficient multiplier; it
    runs every dequant multiply. GpSimd does zero tensor work (shared
    SBUF port halves both engines). Weight TTs fuse two groups at a time
    [128, 2, 4096] against a stride-0-broadcast wsT (measured 4.4us).
  - Per-DMA cost is ~1.5-2us serialized per ring, so DMAs are FEW and
    BIG: the host lays weight groups out by role so each DMA covers two
    groups. SWDGE cast-DMAs land 10 groups as bf16 (incl. both ends of
    the pipeline); ScalarE casts the other 6 (3 fused pair-casts).
  - SP(sync) ring carries only big loads; ACT ring carries the tiny
    prologue + weight scales first; ACT engine never mixes casts with
    big DMA issue.
  - g0 is chunk-granular off quartered SWDGE DMAs (early DVE start);
    g14 is chunk-granular at the tail.

Host-side marshaling stays format-only (LUT decode + layout + dtype cast);
all reference arithmetic (scale multiplies, matmul, bias) is on device.
"""

import json
from contextlib import ExitStack

import ml_dtypes
import numpy as np

import concourse.bass as bass
import concourse.mybir as mybir
import concourse.tile as tile
from concourse.bass_utils import run_bass_kernel_spmd


def _split_multi_waits(m: dict) -> dict:
    """This walrus build allows at most one sync-wait command per instruction.
    Hoist extra waits into standalone EventSemaphore instructions issued just
    before the owning instruction on the same engine queue (semantically
    identical: the engine stalls in order)."""
    for fn in m["functions"]:
        for blk in fn["blocks"]:
            new = []
            ctr = 0
            for inst in blk["instructions"]:
                si = inst.get("sync_info")
                waits = (si or {}).get("on_wait") or []
                if len(waits) > 1:
                    for w in waits[:-1]:
                        new.append({
                            "debug": inst.get("debug", 0),
                            "engine": inst["engine"],
                            "ins": [],
                            "outs": [],
                            "name": f"{inst['name']}-hw{ctr}",
                            "opcode": "EventSemaphore",
                            "sync_info": {"on_update": [], "on_wait": [w]},
                        })
                        ctr += 1
                    si["on_wait"] = [waits[-1]]
                new.append(inst)
            blk["instructions"] = new
    return m


class _SplitWaitBass(bass.Bass):
    def to_json_bytes(self) -> bytes:
        m = json.loads(super().to_json_bytes())
        return json.dumps(_split_multi_waits(m)).encode()


BF16 = ml_dtypes.bfloat16
FP4_LUT = np.array(
    [0.0, 0.5, 1.0, 1.5, 2.0, 3.0, 4.0, 6.0,
     -0.0, -0.5, -1.0, -1.5, -2.0, -3.0, -4.0, -6.0],
    dtype=np.float32,
)

M, K, N = 64, 8192, 8192
NCORES = 8
NS = N // NCORES        # 1024 output columns per core
BLOCK = 16
B = K // BLOCK          # 512 scale blocks along K
P = 128                 # partitions
CHUNKS = K // P         # 64 K-chunks
CB = B // P             # 4 scale-chunk columns (c index)
J = BLOCK               # 16 j-groups (one group = CB chunks = 512 rows)
GW = CB * NS            # 4096 columns per weight group tile
NQ = NS                 # 1024 columns per chunk of a group tile

# Host wvp column order: groups laid out so each DMA covers a contiguous
# role-block. S-block (SWDGE bf16 cast-DMA): g0, then pairs (1,3) (5,7)
# (9,11) (13,15), then g14. A-block (ACT cast): pairs (2,4) (6,8) (10,12).
NEWORD = (0, 1, 3, 5, 7, 9, 11, 13, 15, 14, 2, 4, 6, 8, 10, 12)
POS = {g: i for i, g in enumerate(NEWORD)}
S_PAIRS = ((1, 3), (5, 7), (9, 11), (13, 15))
A_PAIRS = ((2, 4), (6, 8), (10, 12))
# dequant/matmul line order (also PE emission order)
LINE = ("g0", ("S", 1, 3), ("A", 2, 4), ("S", 5, 7), ("A", 6, 8),
        ("S", 9, 11), ("A", 10, 12), ("S", 13, 15), "g14")

_CACHE: dict = {}


def _build_program() -> bass.Bass:
    nc = _SplitWaitBass("TRN2", target_bir_lowering=False, debug=False,
                        num_devices=NCORES)
    dt = mybir.dt

    wvp = nc.dram_tensor("wvp", [P, CHUNKS * NS], dt.float8e4,
                         kind="ExternalInput").ap()
    wst = nc.dram_tensor("wst", [P, GW], dt.bfloat16,
                         kind="ExternalInput").ap()
    xvp = nc.dram_tensor("xvp", [P, CHUNKS * M], dt.bfloat16,
                         kind="ExternalInput").ap()
    xst = nc.dram_tensor("xst", [P, CB * M], dt.bfloat16,
                         kind="ExternalInput").ap()
    gs = nc.dram_tensor("gs", [P, 2], dt.float32, kind="ExternalInput").ap()
    bia = nc.dram_tensor("bia", [1, NS], dt.bfloat16, kind="ExternalInput").ap()
    out = nc.dram_tensor("out", [M, NS], dt.bfloat16, kind="ExternalOutput").ap()

    def wslab(g0pos, ngroups):
        """DRAM slice for `ngroups` consecutive groups starting at layout
        position g0pos."""
        return wvp[:, g0pos * GW:(g0pos + ngroups) * GW]

    with tile.TileContext(nc) as tc, ExitStack() as ctx:
        const = ctx.enter_context(tc.tile_pool(name="const", bufs=1))
        w8pool = ctx.enter_context(tc.tile_pool(name="w8", bufs=1))
        whpool = ctx.enter_context(tc.tile_pool(name="wh", bufs=1))
        ppool = ctx.enter_context(tc.tile_pool(name="acc", bufs=1, space="PSUM"))

        # ---- tiles ----
        wsT = const.tile([P, GW], dt.bfloat16)
        xva = const.tile([P, CHUNKS * M], dt.bfloat16)
        gt = const.tile([P, 2], dt.float32)
        xsT = const.tile([P, CB * M], dt.bfloat16)
        bsb = const.tile([1, NS], dt.bfloat16)
        # bf16 dequant tiles (TT output / SWDGE landing): singles + pairs
        whS0 = whpool.tile([P, GW], dt.bfloat16, name="whS0")
        whA14 = whpool.tile([P, GW], dt.bfloat16, name="whA14")
        pt: dict = {}
        for a, b in S_PAIRS:
            pt[a] = whpool.tile([P, 2 * GW], dt.bfloat16, name=f"pt_{a}")
        for a, b in A_PAIRS:
            pt[a] = whpool.tile([P, 2 * GW], dt.bfloat16, tag="ptA",
                                name=f"pt_{a}", bufs=2)
        # fp8 landing tiles for A-block
        w8: dict = {}
        for a, b in A_PAIRS:
            w8[a] = w8pool.tile([P, 2 * GW], dt.float8e4, name=f"w8_{a}")
        w8[14] = w8pool.tile([P, GW], dt.float8e4, name="w8_14")

        # ---- DMAs (big, few; three rings in parallel) ----
        # ACT/HWDGE ring: wsT first quarter (gates first TT), rest of wsT,
        # tiny x-side tensors. No other ACT-queue work before the casts.
        nc.scalar.dma_start(wsT[:, 0:NQ], wst[:, 0:NQ])
        nc.scalar.dma_start(wsT[:, NQ:GW], wst[:, NQ:GW])
        nc.scalar.dma_start(gt[:], gs[:])
        nc.scalar.dma_start(xsT[:], xst[:])
        nc.scalar.dma_start(bsb[:], bia[:])

        # SWDGE ring: g0 quartered (cast to bf16, chunk-gated early start),
        # then the four S-pairs, then g14's fp8 (cast later by ACT).
        for q in range(CB):
            nc.gpsimd.dma_start(whS0[:, q * NQ:(q + 1) * NQ],
                                wvp[:, q * NQ:(q + 1) * NQ])
        for a, b in S_PAIRS:
            nc.gpsimd.dma_start(pt[a][:], wslab(POS[a], 2))
        nc.gpsimd.dma_start(w8[14][:], wslab(POS[14], 1))

        # SP/HWDGE ring: x codes, then the A-block fp8 pair-slabs.
        nc.sync.dma_start(xva[:], xvp[:])
        for a, b in A_PAIRS:
            nc.sync.dma_start(w8[a][:], wslab(POS[a], 2))

        # ---- x-side dequant ----
        gcol = const.tile([P, 1], dt.float32)
        nc.vector.tensor_mul(gcol[:], gt[:, 0:1], gt[:, 1:2])
        xsb = const.tile([P, CB * M], dt.bfloat16)
        nc.vector.tensor_scalar_mul(xsb[:], xsT[:], gcol[:])
        # xhat[p, (j c m)] = xva * xsb[p, (c m)] broadcast over j
        xhat = const.tile([P, CHUNKS * M], dt.bfloat16)
        xsb_b = xsb[:].unsqueeze(1).broadcast_to([P, J, CB * M])
        nc.vector.tensor_mul(
            xhat[:].rearrange("p (j w) -> p j w", j=J),
            xva[:].rearrange("p (j w) -> p j w", j=J),
            xsb_b,
        )

        ones = const.tile([1, M], dt.bfloat16)
        nc.vector.memset(ones[:], 1.0)

        psum = ppool.tile([M, NS], dt.float32)
        for h in range(2):
            nc.tensor.matmul(
                psum[:, h * 512:(h + 1) * 512],
                ones[:1, :],
                bsb[:1, h * 512:(h + 1) * 512],
                start=True,
                stop=False,
            )

        def emit_mms(g, c, wh, col0, stop=False):
            """8 matmuls for group g chunk c; weight cols start at col0."""
            t = g * CB + c
            for h in range(2):
                nc.tensor.matmul(
                    psum[:, h * 512:(h + 1) * 512],
                    xhat[:, t * M:(t + 1) * M],
                    wh[:, col0 + c * NS + h * 512: col0 + c * NS + (h + 1) * 512],
                    start=False,
                    stop=stop,
                )

        wsT_b2 = wsT[:].unsqueeze(1).broadcast_to([P, 2, GW])

        # ---- g0: SWDGE-landed bf16, chunk-granular TTs (early start) ----
        for c in range(CB):
            cs = slice(c * NQ, (c + 1) * NQ)
            nc.vector.tensor_mul(whS0[:, cs], whS0[:, cs], wsT[:, cs])
            emit_mms(0, c, whS0, 0)

        # ---- middle: alternating S-pairs and A-pairs, fused pair TTs ----
        for kind, a, b in LINE[1:-1]:
            tle = pt[a]
            if kind == "A":
                nc.scalar.copy(tle[:], w8[a][:])        # fused 2-group cast
            nc.vector.tensor_mul(
                tle[:].rearrange("p (q w) -> p q w", q=2),
                tle[:].rearrange("p (q w) -> p q w", q=2),
                wsT_b2,
            )
            for c in range(CB):
                emit_mms(a, c, tle, 0)
            for c in range(CB):
                emit_mms(b, c, tle, GW)

        # ---- g14: ACT chunk-casts + chunk TTs (short tail) ----
        for c in range(CB):
            cs = slice(c * NQ, (c + 1) * NQ)
            nc.scalar.copy(whA14[:, cs], w8[14][:, cs])
            nc.vector.tensor_mul(whA14[:, cs], whA14[:, cs], wsT[:, cs])
            emit_mms(14, c, whA14, 0, stop=(c == CB - 1))

        # ---- tail: PSUM -> SBUF -> HBM ----
        osb = const.tile([M, NS], dt.bfloat16)
        nc.vector.tensor_copy(osb[:, 0:512], psum[:, 0:512])
        nc.scalar.copy(osb[:, 512:NS], psum[:, 512:NS])
        nc.scalar.dma_start(out[:], osb[:])

    return nc


def _perm_k(vals_2d: np.ndarray) -> np.ndarray:
    """[R, K] fp values -> [K, R] with K permuted as r = j*B + b."""
    r = vals_2d.shape[0]
    return (
        vals_2d.reshape(r, B, BLOCK).transpose(2, 1, 0).reshape(K, r)
    )


def _swz(rows_2d: np.ndarray, width: int) -> np.ndarray:
    """[n_chunks*128, width] -> [128, n_chunks*width]: row p holds chunk-major
    data for partition p (per-partition-contiguous DMA layout)."""
    n = rows_2d.shape[0] // P
    return np.ascontiguousarray(
        rows_2d.reshape(n, P, width).transpose(1, 0, 2).reshape(P, n * width)
    )


def prepare_in_maps(**inputs) -> list[dict[str, np.ndarray]]:
    x = np.asarray(inputs["x"]).astype(np.int64)
    xs = np.asarray(inputs["x_scale"], dtype=np.float32)
    gx = np.float32(np.asarray(inputs["x_global_scale"]).reshape(-1)[0])
    w = np.asarray(inputs["weight"]).astype(np.int64)
    ws = np.asarray(inputs["weight_scale"], dtype=np.float32)
    gw = np.float32(np.asarray(inputs["weight_global_scale"]).reshape(-1)[0])
    b = np.asarray(inputs["bias"], dtype=np.float32)

    FP8 = ml_dtypes.float8_e4m3
    xvp = _swz(_perm_k(FP4_LUT[x]).astype(BF16), M)                  # [128, 4096]
    xst = _swz(np.ascontiguousarray(xs.T), M).astype(BF16)           # [128, 256]
    gs = np.tile(np.array([[gx, gw]], dtype=np.float32), (P, 1))

    wv = FP4_LUT[w]                                                  # [N, K] f32
    in_maps = []
    for c in range(NCORES):
        sl = slice(c * NS, (c + 1) * NS)
        wvp = _swz(_perm_k(wv[sl]).astype(FP8), NS)                  # [128, 64*NS]
        # permute the group blocks into role order (NEWORD)
        wg = wvp.reshape(P, J, GW)
        wvp_r = np.ascontiguousarray(
            wg[:, list(NEWORD), :].reshape(P, J * GW))
        in_maps.append({
            "wvp": wvp_r,
            "wst": _swz(ws[sl].T.astype(BF16), NS),                  # [128, 4*NS]
            "xvp": xvp,
            "xst": xst,
            "gs": gs,
            "bia": np.ascontiguousarray(b[sl].reshape(1, NS)).astype(BF16),
        })
    return in_maps


LAST_RESULTS = None


def kernel(**inputs) -> np.ndarray:
    global LAST_RESULTS
    if "nc" not in _CACHE:
        _CACHE["nc"] = _build_program()
    nc = _CACHE["nc"]

    in_maps = prepare_in_maps(**inputs)
    res = run_bass_kernel_spmd(nc, in_maps, core_ids=list(range(NCORES)))
    LAST_RESULTS = res
    out = np.concatenate([res.results[c]["out"] for c in range(NCORES)], axis=1)
    return out.astype(BF16)
